# revision 59
# baseline (speedup 1.0000x reference)
"""Trainium2 Bass kernel for nn_KB_Mapping_19361712570541 (dense_cnn).

Math (W=1 image dim folded away; h = x.T in [C, N] channels-on-partition):
  dw3(h, w)[c,n] = w0[c]h[c,n-1] + w1[c]h[c,n] + w2[c]h[c,n+1]   (zero pad)
  b1  = relu(W1pw @ relu(dw3(h, wd1)))
  b2  = (relu(W21x1 @ h) + b1) * mask
  b2  = relu(W2pw @ relu(dw3(b2, wd2)))
  out = relu(Wf[:, :C] @ h + Wf[:, C:] @ b2)          -> out.T is [N, C]

Sharding: data-parallel along N across 8 cores; each core's input slab
carries halos of x/mask so no cross-core communication is needed.

Implementation notes (cost-model driven):
- The kernel is PSUM-evacuation bound: every relu forces one PSUM->SBUF
  pointwise op, PSUM is only reachable from DVE (658ns/512 cols) and
  ACT (612ns/512; 1039ns/1024 across two banks) - GPSIMD cannot access
  PSUM, and fp32 PSUM operands disable all DVE fast modes.
- The mask multiply is FOLDED into the relu evacuations that feed dw2:
  u0 = relu(b1p)*m, u1 = relu(a)*m (valid since m>=0); dw2 consumes
  u0+u1 linearly via three paired DoubleRow passes (plane stride 516
  inside one SBUF tile). b1p and a are matmul'd into ONE [C,1024] PSUM
  tile (2 adjacent banks) and evacuated by a single DVE
  scalar_tensor_tensor whose input AP spans both banks, output planes
  stride 516 into u, and whose mask AP uses plane-stride 0 (broadcast).
- d1p(j) and d2p(j-2) (independent pipeline stages) share one
  double-buffered [C,1024] PSUM pair and are evacuated by ONE wide ACT
  relu into a shared dd tile (d1s at [0:516], d2s at [516:1032]).
- Per tile: DVE = combo 1192 + 5/6 x 658 (b2r), ACT = wide 1039 +
  612 (out) + 1/6 x 612 -> ~1.75us/tile on both engines.
- All matmuls are fp8e4 DoubleRow (cost = out cols x 0.208ns). x ships
  ONCE as fp8 (fusion's Wfh@h term also fp8; rel err ~5e-3 vs 2e-2
  gate), mask as fp8, y as fp16: ~25us DMA per core.
- PSUM banks: dd_p x2 pairs (4), ba pair (2), b2 (1), f (1) = 8.
  Zero-weight DR planes read 2 cols past valid data; work-tile rings
  are memset once per buffer at startup (interp faults on uninit).
- Software-pipelined emission, iteration i:
  A(i) dw1 MMs; C(i-2) dw2 MMs; W(i) wide dd evac; B(i-1) b1p/a MMs +
  combo; E(i-4) out relu + flush; D(i-3) b2p MM + b2r + fusion MMs.
  (E before D so the single-bank f is freed before the next fusion.)
"""

import numpy as np
from contextlib import ExitStack

import ml_dtypes

import concourse.bass as bass
import concourse.bacc as bacc
import concourse.tile as tile
import concourse.mybir as mybir
from concourse.ap import AP
from concourse.bass_utils import run_bass_kernel_spmd

C = 128
N = 131072
NCORES = 8
NSH = N // NCORES          # 16384 output columns per core
T = 510                    # steady-state tile width (wE = 512 = one PSUM bank)
MASK_SEED = 42
MASK_P = 0.5

F32 = mybir.dt.float32
F16 = mybir.dt.float16
F8 = mybir.dt.float8e4
NP8 = ml_dtypes.float8_e4m3
DR = mybir.MatmulPerfMode.DoubleRow
Relu = mybir.ActivationFunctionType.Relu

# DR weight-pair indices in w8 (each pair is [C, 2, C] -> 256 cols)
(P_D1A, P_D1B, P_W1, P_W21, P_D2K0, P_D2K1, P_D2K2, P_W2, P_WFH,
 P_WFB, P_WFHR) = range(11)
NPAIR = 11

LAST_RESULT = None         # BassKernelResults of the most recent run (for test.py)
TRACE = False

_mask_cache = None
_nc_cache = None


def _mask_cn() -> np.ndarray:
    """The reference's fixed Bernoulli mask in [C, N] layout, float32."""
    global _mask_cache
    if _mask_cache is None:
        import jax
        cpu = jax.devices("cpu")[0]
        with jax.default_device(cpu):
            m = jax.random.bernoulli(
                jax.random.key(MASK_SEED), 1.0 - MASK_P, (1, C, N, 1)
            )
            _mask_cache = np.asarray(m)[0, :, :, 0].astype(np.float32)
    return _mask_cache


def _tiles():
    """(a, width) list covering [0, NSH); narrow leader fills the pipe and
    a narrow trailer drains it."""
    widths = [192] + [T] * 31 + [382]
    assert sum(widths) == NSH
    out, a = [], 0
    for w in widths:
        out.append((a, w))
        a += w
    return out


def _groups(tiles):
    """Graduated DMA groups as slices of the tile list."""
    sizes = [4, 5, 7, 8, 9]
    gs, i = [], 0
    for s in sizes:
        if i >= len(tiles):
            break
        gs.append(tiles[i:i + s])
        i += s
    if i < len(tiles):
        gs.append(tiles[i:])
    return gs


def _dr_rhs(t, col, n, delta=2):
    """[C, 2, n] moving AP over tile t: plane0 at col, plane1 at col+delta."""
    base = t[:, col:col + n]
    return AP(base.tensor, base.offset,
              [list(base.ap[0]), [delta, 2], [1, n]])


def _planes(t, col, n, stride):
    """[C, 2, n] AP over tile t: planes at col and col+stride."""
    base = t[:, col:col + n]
    return AP(base.tensor, base.offset,
              [list(base.ap[0]), [stride, 2], [1, n]])


def _build_nc():
    nc = bacc.Bacc("TRN2", target_bir_lowering=False)

    # x8 holds the fp8 input AND its fp8 quantization residual as a second
    # plane: x8[:, 0:NSH+8] = fp8(x), x8[:, NSH+8:] = fp8(x - fp8(x)).
    # The residual rides the otherwise-zero second weight plane of the
    # fusion / b2a / dw1-t1 DoubleRow passes, recovering ~fp16 accuracy on
    # the direct paths at zero extra PE cost.
    x8 = nc.dram_tensor("x8", [C, 2 * (NSH + 8)], F8, kind="ExternalInput")
    mk = nc.dram_tensor("mk", [C, NSH + 2], F8, kind="ExternalInput")
    w8 = nc.dram_tensor("w8", [C, NPAIR * 2 * C], F8, kind="ExternalInput")
    y = nc.dram_tensor("y", [C, NSH], F16, kind="ExternalOutput")

    tiles = _tiles()
    groups = _groups(tiles)
    gw_max = max(sum(w for _, w in g) for g in groups)
    S = gw_max + 6            # xs slab plane stride (x -> residual)
    group_of = {}
    for gi, g in enumerate(groups):
        for t_ in g:
            group_of[t_[0]] = gi

    with ExitStack() as ctx:
        tc = ctx.enter_context(tile.TileContext(nc))
        wpool = ctx.enter_context(tc.tile_pool(name="weights", bufs=1))
        slab = ctx.enter_context(tc.tile_pool(name="slab", bufs=3))
        opool = ctx.enter_context(tc.tile_pool(name="out", bufs=3))
        work = ctx.enter_context(tc.tile_pool(name="work", bufs=3))
        ps = ctx.enter_context(tc.tile_pool(name="ps", bufs=1, space="PSUM"))

        w8_sb = wpool.tile([C, NPAIR * 2 * C], F8)

        def wpair(k):
            return w8_sb[:, k * 2 * C:(k + 1) * 2 * C].rearrange(
                "p (two m) -> p two m", two=2)

        def relu_to(eng, dst, src):
            if eng == "A":
                nc.scalar.activation(dst, src, Relu)
            elif eng == "S":
                # drain helper: both engines in parallel on half-widths
                w = dst.shape[-1]
                h = min(int(w * 0.51), w)
                nc.vector.tensor_scalar_max(dst[:, :h], src[:, :h], 0.0)
                nc.scalar.activation(dst[:, h:], src[:, h:], Relu)
            else:
                nc.vector.tensor_scalar_max(dst, src, 0.0)

        # One-time zero of every ring buffer that zero-weight DR planes or
        # edge-width wide evacs read past valid data on (the interpreter
        # hard-faults on uninitialized reads). Runs on Pool/DVE during the
        # initial DMA fill, off the critical path.
        # preload the ACT Relu table before any real data arrives: the
        # one-time LoadActFuncSet (1283ns) otherwise blocks the first
        # wide evacuation mid-fill.
        warm = wpool.tile([C, 4], F16)
        nc.gpsimd.memset(warm[:, :], 0.0)
        nc.scalar.activation(warm[:, 2:4], warm[:, 0:2], Relu)

        ZRINGS = (("dd", 4, 1032), ("u", 4, 1032), ("b2r", 3, 516))
        ztiles = {ztag: [work.tile([C, zw], F8, tag=ztag, bufs=zb,
                                   name=f"z_{ztag}")
                         for _ in range(zb)]
                  for ztag, zb, zw in ZRINGS}
        # earliest-used buffers first, split across Pool and DVE so the
        # first tiles' writes aren't serialized behind one engine's queue
        zi = 0
        for bi in range(3):
            for ztag, zb, zw in ZRINGS:
                if bi < zb:
                    zt = ztiles[ztag][bi]
                    if zi % 2 == 0:
                        nc.gpsimd.memset(zt[:, :], 0.0)
                    else:
                        nc.vector.memset(zt[:, :], 0.0)
                    zi += 1

        # per-group slab state
        cur = {}

        def load_group(gi):
            g = groups[gi]
            ga = g[0][0]
            gw = sum(w for _, w in g)
            xs = slab.tile([C, 2 * S], F8, tag="xs")
            nc.sync.dma_start(
                out=_planes(xs, 0, gw + 6, S),
                in_=_planes(x8, ga, gw + 6, NSH + 8))
            ms = slab.tile([C, gw_max + 2], F8, tag="ms")
            if gi == 0:
                # fill-critical order: dw1 pairs -> first two tiles' mask
                # -> W1/W21 pairs -> rest of mask -> back-pipeline pairs
                # (dw2/W2/Wf, needed only ~LAG iterations in).
                nc.sync.dma_start(out=w8_sb[:, :512], in_=w8[:, :512])
                nc.sync.dma_start(out=ms[:, :772], in_=mk[:, :772])
                nc.sync.dma_start(out=w8_sb[:, 512:2048],
                                  in_=w8[:, 512:2048])
                nc.sync.dma_start(out=ms[:, 772:gw + 2],
                                  in_=mk[:, 772:gw + 2])
                nc.sync.dma_start(out=w8_sb[:, 2048:], in_=w8[:, 2048:])
            else:
                nc.sync.dma_start(out=ms[:, :gw + 2],
                                  in_=mk[:, ga:ga + gw + 2])
            o_c = opool.tile([C, gw_max], F16, tag="oc")
            cur[gi] = dict(ga=ga, gw=gw, xs=xs, ms=ms, o_c=o_c,
                           flushed=0, done=0)

        def stage_A(st):
            """dw1 MMs into the d1 half of this tile's dd_p pair."""
            g = cur[st["gi"]]
            la, wE = st["la"], st["wE"]
            xs = g["xs"]
            dd_p = ps.tile([C, 1024], F32, tag="ddp", bufs=2, name="dd_p")
            nc.tensor.matmul(dd_p[:, :wE], wpair(P_D1A),
                             _dr_rhs(xs, la, wE),
                             start=True, stop=False, perf_mode=DR)
            nc.tensor.matmul(dd_p[:, :wE], wpair(P_D1B),
                             _planes(xs, la + 1, wE, S),
                             start=False, stop=True, perf_mode=DR)
            st.update(dd_p=dd_p)

        def stage_C(st, host):
            """dw2 MMs (3 paired DR passes over u0,u1) into the d2 half of
            the CURRENT iteration's dd_p (host st)."""
            wC = st["wE"]  # d2 computed at the same width as the front ops
            u = st["u"]
            dd_p = host["dd_p"]
            for k, pk in enumerate((P_D2K0, P_D2K1, P_D2K2)):
                nc.tensor.matmul(dd_p[:, 512:512 + wC], wpair(pk),
                                 _planes(u, k, wC, 516),
                                 start=(k == 0), stop=(k == 2), perf_mode=DR)
            st.update(dd_host=host)

        def stage_W(st_a, st_c):
            """One wide relu evac covering d1p(i) and d2p(i-LAG); narrow
            single evacs at the pipeline edges. Rotates DVE/ACT."""
            if st_a is not None:
                dd = work.tile([C, 1032], F8, tag="dd", bufs=4)
                st_a.update(dd=dd)
            if st_a is not None and st_c is not None and st_a["wE"] == 512 \
                    and st_c["wE"] == 512:
                dd_p = st_a["dd_p"]
                out_ap = _planes(dd, 0, 512, 516)
                in_ap = _planes(dd_p, 0, 512, 512)
                if st_a["e_w"] == "A":
                    nc.scalar.activation(out_ap, in_ap, Relu)
                else:
                    nc.vector.tensor_scalar_max(out_ap, in_ap, 0.0)
                st_c.update(d2s=dd)
                return
            if st_a is not None:
                wE = st_a["wE"]
                relu_to("A", dd[:, :wE], st_a["dd_p"][:, :wE])
            if st_c is not None:
                wC = st_c["wE"]
                host = st_c["dd_host"]
                dd_c = host.get("dd")
                relu_to("A", dd_c[:, 516:516 + wC],
                        host["dd_p"][:, 512:512 + wC])
                st_c.update(d2s=dd_c)

        def stage_B(st):
            """b1p + a MMs into the 2-bank ba tile -> combo STT -> u
            (u0 = relu(b1p)*m, u1 = relu(a)*m in one DVE op; the mask AP
            broadcasts with plane-stride 0)."""
            g = cur[st["gi"]]
            la, wE = st["la"], st["wE"]
            ba = ps.tile([C, 1024], F32, tag="ba", name="ba")
            nc.tensor.matmul(ba[:, :wE], wpair(P_W1),
                             _dr_rhs(st["dd"], 0, wE),
                             start=True, stop=True, perf_mode=DR)
            nc.tensor.matmul(ba[:, 512:512 + wE], wpair(P_W21),
                             _planes(g["xs"], la + 1, wE, S),
                             start=True, stop=True, perf_mode=DR)
            u = work.tile([C, 1032], F8, tag="u", bufs=4)
            out_ap = _planes(u, 0, wE, 516)
            in_ap = _planes(ba, 0, wE, 512)
            mb = g["ms"][:, la:la + wE]
            m_ap = AP(mb.tensor, mb.offset,
                      [list(mb.ap[0]), [0, 2], [1, wE]])
            nc.vector.scalar_tensor_tensor(
                out_ap, in_ap, 0.0, m_ap,
                mybir.AluOpType.max, mybir.AluOpType.mult)
            st.update(u=u)

        def stage_D1(st):
            """b2p MM -> b2r relu. The fusion MMs run one iteration later
            (stage_D2) so no PE instruction ever waits on a same-iteration
            evacuation."""
            dd_c = st["d2s"]
            P_ = st["P_"]
            b2p = ps.tile([C, 512], F32, tag="b2", name="b2p")
            nc.tensor.matmul(b2p[:, :P_], wpair(P_W2),
                             _dr_rhs(dd_c, 516, P_),
                             start=True, stop=True, perf_mode=DR)
            b2r = work.tile([C, 516], F8, tag="b2r", bufs=3)
            relu_to(st["e_b2r"], b2r[:, :P_], b2p[:, :P_])
            st.update(b2r=b2r)

        def stage_D2(st):
            """fusion MMs; Wfb first (its b2r is one iteration old), then
            Wfh + the x/Wfh residual planes close the group."""
            g = cur[st["gi"]]
            la, P_ = st["la"], st["P_"]
            fp = ps.tile([C, 512], F32, tag="f", name="fp")
            nc.tensor.matmul(fp[:, :P_], wpair(P_WFB),
                             _dr_rhs(st["b2r"], 0, P_),
                             start=True, stop=False, perf_mode=DR)
            nc.tensor.matmul(fp[:, :P_], wpair(P_WFH),
                             _planes(g["xs"], la + 2, P_, S),
                             start=False, stop=False, perf_mode=DR)
            # fp8 residual of the Wfh weight itself: the direct h->out path
            # is unattenuated, so Wfh's own quantization error (~2% rel)
            # must be compensated too.
            nc.tensor.matmul(fp[:, :P_], wpair(P_WFHR),
                             _planes(g["xs"], la + 2, P_, S),
                             start=False, stop=True, perf_mode=DR)
            st.update(fp=fp)

        def stage_E(st):
            """final relu + output flush bookkeeping."""
            g = cur[st["gi"]]
            la, P_ = st["la"], st["P_"]
            relu_to(st["e_out"], g["o_c"][:, la:la + P_], st["fp"][:, :P_])
            g["done"] += 1
            ntiles = len(groups[st["gi"]])
            # flush every 2 finished tiles
            if g["done"] % 2 == 0 or g["done"] == ntiles:
                lo, hi = g["flushed"], la + P_
                nc.sync.dma_start(out=y[:, g["ga"] + lo:g["ga"] + hi],
                                  in_=g["o_c"][:, lo:hi])
                g["flushed"] = hi

        flat = [t_ for g in groups for t_ in g]
        n = len(flat)
        sts = []
        loaded = 0

        def ensure_loaded(upto):
            nonlocal loaded
            while loaded <= min(upto, len(groups) - 1):
                load_group(loaded)
                loaded += 1

        # Pipeline skew (iteration i):
        #   A(i) dw1 -> dd_p(i)[0:512]
        #   C(i-LAG) dw2 -> dd_p(i)[512:1024]   (u from B(i-LAG), 2 iters old)
        #   W(i) wide relu dd_p(i) -> d1s(i), d2s(i-LAG)
        #   B(i-2) b1p/a MMs + combo STT -> u
        #   E(i-LAG-4) out relu + flush   (before D2: frees the f bank)
        #   D2(i-LAG-3) fusion MMs        (b2r one iteration old)
        #   D1(i-LAG-2) b2p MM + b2r relu
        LAG = 4
        ensure_loaded(0)
        for i in range(n + LAG + 4):
            st_a = None
            if i < n:
                a, P_ = flat[i]
                gi = group_of[a]
                ensure_loaded(gi + 1)
                # LP balance: DVE {combo + single on 5-of-6 tiles} ~1739,
                # ACT {dd-wide + singles} ~1753; on the 1-in-6 "light"
                # tile DVE runs only the combo and ACT absorbs all three
                # remaining ops (the 2-iteration slack edges soak it up).
                light = (i % 6 == 5)
                st = dict(a=a, P_=P_, wE=P_ + 2, gi=gi,
                          la=a - cur[gi]["ga"],
                          e_w="A",
                          e_b2r=("A" if light else "D"),
                          e_out="A")
                sts.append(st)
                st_a = st
                stage_A(st)
            st_c = sts[i - LAG] if 0 <= i - LAG < n else None
            if st_c is not None:
                host = st_a if st_a is not None else None
                if host is None:
                    # drain iterations: d2 half still needs a PSUM pair;
                    # allocate a fresh one for the trailing tiles
                    dd_p_t = ps.tile([C, 1024], F32, tag="ddp", bufs=2,
                                     name="dd_p_t")
                    dd_t = work.tile([C, 1032], F8, tag="dd", bufs=4,
                                     name="dd_t")
                    host = dict(dd_p=dd_p_t, dd=dd_t)
                stage_C(st_c, host)
            stage_W(st_a, st_c)
            if 0 <= i - 2 < n:
                stage_B(sts[i - 2])
            if 0 <= i - LAG - 4 < n:
                stage_E(sts[i - LAG - 4])
            if 0 <= i - LAG - 3 < n:
                stage_D2(sts[i - LAG - 3])
            if 0 <= i - LAG - 2 < n:
                stage_D1(sts[i - LAG - 2])

    nc.compile()
    return nc


def kernel(x, w_b1_dw, w_b1_pw, w_b2_1x1, w_b2_dw, w_b2_pw, w_fusion):
    global LAST_RESULT, _nc_cache

    x = np.asarray(x, dtype=np.float32)
    h = np.ascontiguousarray(x.T)
    mask = _mask_cn()

    # host-side shard prep: [C, N] layouts, zero-padded halos; second
    # plane is the fp8 quantization residual of x (see _build_nc notes)
    x8_pad = np.zeros((C, N + 8), dtype=NP8)
    x8_pad[:, 2:N + 2] = h.astype(NP8)
    r8_pad = np.zeros((C, N + 8), dtype=NP8)
    r8_pad[:, 2:N + 2] = (h - x8_pad[:, 2:N + 2].astype(np.float32)
                          ).astype(NP8)
    mk_pad = np.zeros((C, N + 2), dtype=NP8)
    mk_pad[:, 1:N + 1] = mask.astype(NP8)

    def taps(wdw):  # [C,1,3,3] -> per-channel taps along N
        return np.asarray(wdw)[:, 0, :, 1]  # [C, 3]

    t1 = taps(w_b1_dw)
    t2 = taps(w_b2_dw)

    def diag8(v):
        return np.diag(v.astype(np.float32)).astype(NP8)

    def lhsT8(w):  # [O, I] -> [I, O] fp8
        return np.ascontiguousarray(np.asarray(w, dtype=np.float32).T).astype(NP8)

    zero = np.zeros((C, C), dtype=NP8)
    w21T = lhsT8(np.asarray(w_b2_1x1)[:, :, 0, 0])
    wfh_f32 = np.ascontiguousarray(
        np.asarray(w_fusion, dtype=np.float32)[:, :C, 0, 0].T)
    wfhT = wfh_f32.astype(NP8)
    wfhrT = (wfh_f32 - wfhT.astype(np.float32)).astype(NP8)
    pairs = [
        (diag8(t1[:, 0]), diag8(t1[:, 2])),
        (diag8(t1[:, 1]), diag8(t1[:, 1])),   # second plane: x residual
        (lhsT8(np.asarray(w_b1_pw)[:, :, 0, 0]), zero),
        (w21T, w21T),                          # second plane: x residual
        (diag8(t2[:, 0]), diag8(t2[:, 0])),
        (diag8(t2[:, 1]), diag8(t2[:, 1])),
        (diag8(t2[:, 2]), diag8(t2[:, 2])),
        (lhsT8(np.asarray(w_b2_pw)[:, :, 0, 0]), zero),
        (wfhT, wfhT),                          # second plane: x residual
        (lhsT8(np.asarray(w_fusion)[:, C:, 0, 0]), zero),
        (wfhrT, wfhrT),                        # Wfh weight residual
    ]
    w8_host = np.empty((C, NPAIR * 2 * C), dtype=NP8)
    for k, (p0, p1) in enumerate(pairs):
        w8_host[:, (2 * k) * C:(2 * k + 1) * C] = p0
        w8_host[:, (2 * k + 1) * C:(2 * k + 2) * C] = p1

    in_maps = []
    for i in range(NCORES):
        s = i * NSH
        x8_core = np.empty((C, 2 * (NSH + 8)), dtype=NP8)
        x8_core[:, :NSH + 8] = x8_pad[:, s:s + NSH + 8]
        x8_core[:, NSH + 8:] = r8_pad[:, s:s + NSH + 8]
        in_maps.append({
            "x8": x8_core,
            "mk": np.ascontiguousarray(mk_pad[:, s:s + NSH + 2]),
            "w8": w8_host,
        })

    if _nc_cache is None:
        _nc_cache = _build_nc()

    res = run_bass_kernel_spmd(
        _nc_cache, in_maps, core_ids=list(range(NCORES)), trace=TRACE
    )
    LAST_RESULT = res

    out = np.empty((C, N), dtype=np.float32)
    for i in range(NCORES):
        out[:, i * NSH:(i + 1) * NSH] = res.results[i]["y"].astype(np.float32)
    return np.ascontiguousarray(out.T)


# revision 60
# speedup vs baseline: 1.0120x; 1.0120x over previous
"""Trainium2 Bass kernel for nn_KB_Mapping_19361712570541 (dense_cnn).

Math (W=1 image dim folded away; h = x.T in [C, N] channels-on-partition):
  dw3(h, w)[c,n] = w0[c]h[c,n-1] + w1[c]h[c,n] + w2[c]h[c,n+1]   (zero pad)
  b1  = relu(W1pw @ relu(dw3(h, wd1)))
  b2  = (relu(W21x1 @ h) + b1) * mask
  b2  = relu(W2pw @ relu(dw3(b2, wd2)))
  out = relu(Wf[:, :C] @ h + Wf[:, C:] @ b2)          -> out.T is [N, C]

Sharding: data-parallel along N across 8 cores; each core's input slab
carries halos of x/mask so no cross-core communication is needed.

Implementation notes (cost-model driven):
- The kernel is PSUM-evacuation bound: every relu forces one PSUM->SBUF
  pointwise op, PSUM is only reachable from DVE (658ns/512 cols) and
  ACT (612ns/512; 1039ns/1024 across two banks) - GPSIMD cannot access
  PSUM, and fp32 PSUM operands disable all DVE fast modes.
- The mask multiply is FOLDED into the relu evacuations that feed dw2:
  u0 = relu(b1p)*m, u1 = relu(a)*m (valid since m>=0); dw2 consumes
  u0+u1 linearly via three paired DoubleRow passes (plane stride 516
  inside one SBUF tile). b1p and a are matmul'd into ONE [C,1024] PSUM
  tile (2 adjacent banks) and evacuated by a single DVE
  scalar_tensor_tensor whose input AP spans both banks, output planes
  stride 516 into u, and whose mask AP uses plane-stride 0 (broadcast).
- d1p(j) and d2p(j-2) (independent pipeline stages) share one
  double-buffered [C,1024] PSUM pair and are evacuated by ONE wide ACT
  relu into a shared dd tile (d1s at [0:516], d2s at [516:1032]).
- Per tile: DVE = combo 1192 + 5/6 x 658 (b2r), ACT = wide 1039 +
  612 (out) + 1/6 x 612 -> ~1.75us/tile on both engines.
- All matmuls are fp8e4 DoubleRow (cost = out cols x 0.208ns). x ships
  ONCE as fp8 (fusion's Wfh@h term also fp8; rel err ~5e-3 vs 2e-2
  gate), mask as fp8, y as fp16: ~25us DMA per core.
- PSUM banks: dd_p x2 pairs (4), ba pair (2), b2 (1), f (1) = 8.
  Zero-weight DR planes read 2 cols past valid data; work-tile rings
  are memset once per buffer at startup (interp faults on uninit).
- Software-pipelined emission, iteration i:
  A(i) dw1 MMs; C(i-2) dw2 MMs; W(i) wide dd evac; B(i-1) b1p/a MMs +
  combo; E(i-4) out relu + flush; D(i-3) b2p MM + b2r + fusion MMs.
  (E before D so the single-bank f is freed before the next fusion.)
"""

import numpy as np
from contextlib import ExitStack

import ml_dtypes

import concourse.bass as bass
import concourse.bacc as bacc
import concourse.tile as tile
import concourse.mybir as mybir
from concourse.ap import AP
from concourse.bass_utils import run_bass_kernel_spmd

C = 128
N = 131072
NCORES = 8
NSH = N // NCORES          # 16384 output columns per core
T = 510                    # steady-state tile width (wE = 512 = one PSUM bank)
MASK_SEED = 42
MASK_P = 0.5

F32 = mybir.dt.float32
F16 = mybir.dt.float16
F8 = mybir.dt.float8e4
NP8 = ml_dtypes.float8_e4m3
DR = mybir.MatmulPerfMode.DoubleRow
Relu = mybir.ActivationFunctionType.Relu

# DR weight-pair indices in w8 (each pair is [C, 2, C] -> 256 cols)
(P_D1A, P_D1B, P_W1, P_W21, P_D2K0, P_D2K1, P_D2K2, P_W2, P_WFH,
 P_WFB, P_WFHR) = range(11)
NPAIR = 11

LAST_RESULT = None         # BassKernelResults of the most recent run (for test.py)
TRACE = False

_mask_cache = None
_nc_cache = None


def _mask_cn() -> np.ndarray:
    """The reference's fixed Bernoulli mask in [C, N] layout, float32."""
    global _mask_cache
    if _mask_cache is None:
        import jax
        cpu = jax.devices("cpu")[0]
        with jax.default_device(cpu):
            m = jax.random.bernoulli(
                jax.random.key(MASK_SEED), 1.0 - MASK_P, (1, C, N, 1)
            )
            _mask_cache = np.asarray(m)[0, :, :, 0].astype(np.float32)
    return _mask_cache


def _tiles():
    """(a, width) list covering [0, NSH); narrow leader fills the pipe and
    a narrow trailer drains it."""
    widths = [320] + [T] * 31 + [254]
    assert sum(widths) == NSH
    out, a = [], 0
    for w in widths:
        out.append((a, w))
        a += w
    return out


def _groups(tiles):
    """Graduated DMA groups as slices of the tile list."""
    sizes = [4, 5, 7, 8, 9]
    gs, i = [], 0
    for s in sizes:
        if i >= len(tiles):
            break
        gs.append(tiles[i:i + s])
        i += s
    if i < len(tiles):
        gs.append(tiles[i:])
    return gs


def _dr_rhs(t, col, n, delta=2):
    """[C, 2, n] moving AP over tile t: plane0 at col, plane1 at col+delta."""
    base = t[:, col:col + n]
    return AP(base.tensor, base.offset,
              [list(base.ap[0]), [delta, 2], [1, n]])


def _planes(t, col, n, stride):
    """[C, 2, n] AP over tile t: planes at col and col+stride."""
    base = t[:, col:col + n]
    return AP(base.tensor, base.offset,
              [list(base.ap[0]), [stride, 2], [1, n]])


def _build_nc():
    nc = bacc.Bacc("TRN2", target_bir_lowering=False)

    # x8 holds the fp8 input AND its fp8 quantization residual as a second
    # plane: x8[:, 0:NSH+8] = fp8(x), x8[:, NSH+8:] = fp8(x - fp8(x)).
    # The residual rides the otherwise-zero second weight plane of the
    # fusion / b2a / dw1-t1 DoubleRow passes, recovering ~fp16 accuracy on
    # the direct paths at zero extra PE cost.
    x8 = nc.dram_tensor("x8", [C, 2 * (NSH + 8)], F8, kind="ExternalInput")
    mk = nc.dram_tensor("mk", [C, NSH + 2], F8, kind="ExternalInput")
    w8 = nc.dram_tensor("w8", [C, NPAIR * 2 * C], F8, kind="ExternalInput")
    y = nc.dram_tensor("y", [C, NSH], F16, kind="ExternalOutput")

    tiles = _tiles()
    groups = _groups(tiles)
    gw_max = max(sum(w for _, w in g) for g in groups)
    S = gw_max + 6            # xs slab plane stride (x -> residual)
    group_of = {}
    for gi, g in enumerate(groups):
        for t_ in g:
            group_of[t_[0]] = gi

    with ExitStack() as ctx:
        tc = ctx.enter_context(tile.TileContext(nc))
        wpool = ctx.enter_context(tc.tile_pool(name="weights", bufs=1))
        slab = ctx.enter_context(tc.tile_pool(name="slab", bufs=3))
        opool = ctx.enter_context(tc.tile_pool(name="out", bufs=3))
        work = ctx.enter_context(tc.tile_pool(name="work", bufs=3))
        ps = ctx.enter_context(tc.tile_pool(name="ps", bufs=1, space="PSUM"))

        w8_sb = wpool.tile([C, NPAIR * 2 * C], F8)

        def wpair(k):
            return w8_sb[:, k * 2 * C:(k + 1) * 2 * C].rearrange(
                "p (two m) -> p two m", two=2)

        def relu_to(eng, dst, src):
            if eng == "A":
                nc.scalar.activation(dst, src, Relu)
            elif eng == "S":
                # drain helper: both engines in parallel on half-widths
                w = dst.shape[-1]
                h = min(int(w * 0.51), w)
                nc.vector.tensor_scalar_max(dst[:, :h], src[:, :h], 0.0)
                nc.scalar.activation(dst[:, h:], src[:, h:], Relu)
            else:
                nc.vector.tensor_scalar_max(dst, src, 0.0)

        # One-time zero of every ring buffer that zero-weight DR planes or
        # edge-width wide evacs read past valid data on (the interpreter
        # hard-faults on uninitialized reads). Runs on Pool/DVE during the
        # initial DMA fill, off the critical path.
        # preload the ACT Relu table before any real data arrives: the
        # one-time LoadActFuncSet (1283ns) otherwise blocks the first
        # wide evacuation mid-fill.
        warm = wpool.tile([C, 4], F16)
        nc.gpsimd.memset(warm[:, :], 0.0)
        nc.scalar.activation(warm[:, 2:4], warm[:, 0:2], Relu)

        ZRINGS = (("dd", 4, 1032), ("u", 4, 1032), ("b2r", 3, 516))
        ztiles = {ztag: [work.tile([C, zw], F8, tag=ztag, bufs=zb,
                                   name=f"z_{ztag}")
                         for _ in range(zb)]
                  for ztag, zb, zw in ZRINGS}
        # earliest-used buffers first, split across Pool and DVE so the
        # first tiles' writes aren't serialized behind one engine's queue
        zi = 0
        for bi in range(3):
            for ztag, zb, zw in ZRINGS:
                if bi < zb:
                    zt = ztiles[ztag][bi]
                    if zi % 2 == 0:
                        nc.gpsimd.memset(zt[:, :], 0.0)
                    else:
                        nc.vector.memset(zt[:, :], 0.0)
                    zi += 1

        # per-group slab state
        cur = {}

        def load_group(gi):
            g = groups[gi]
            ga = g[0][0]
            gw = sum(w for _, w in g)
            xs = slab.tile([C, 2 * S], F8, tag="xs")
            nc.sync.dma_start(
                out=_planes(xs, 0, gw + 6, S),
                in_=_planes(x8, ga, gw + 6, NSH + 8))
            ms = slab.tile([C, gw_max + 2], F8, tag="ms")
            if gi == 0:
                # fill-critical order: dw1 pairs -> first two tiles' mask
                # -> W1/W21 pairs -> rest of mask -> back-pipeline pairs
                # (dw2/W2/Wf, needed only ~LAG iterations in).
                nc.sync.dma_start(out=w8_sb[:, :512], in_=w8[:, :512])
                nc.sync.dma_start(out=ms[:, :772], in_=mk[:, :772])
                nc.sync.dma_start(out=w8_sb[:, 512:2048],
                                  in_=w8[:, 512:2048])
                nc.sync.dma_start(out=ms[:, 772:gw + 2],
                                  in_=mk[:, 772:gw + 2])
                nc.sync.dma_start(out=w8_sb[:, 2048:], in_=w8[:, 2048:])
            else:
                nc.sync.dma_start(out=ms[:, :gw + 2],
                                  in_=mk[:, ga:ga + gw + 2])
            o_c = opool.tile([C, gw_max], F16, tag="oc")
            cur[gi] = dict(ga=ga, gw=gw, xs=xs, ms=ms, o_c=o_c,
                           flushed=0, done=0)

        def stage_A(st):
            """dw1 MMs into the d1 half of this tile's dd_p pair."""
            g = cur[st["gi"]]
            la, wE = st["la"], st["wE"]
            xs = g["xs"]
            dd_p = ps.tile([C, 1024], F32, tag="ddp", bufs=2, name="dd_p")
            nc.tensor.matmul(dd_p[:, :wE], wpair(P_D1A),
                             _dr_rhs(xs, la, wE),
                             start=True, stop=False, perf_mode=DR)
            nc.tensor.matmul(dd_p[:, :wE], wpair(P_D1B),
                             _planes(xs, la + 1, wE, S),
                             start=False, stop=True, perf_mode=DR)
            st.update(dd_p=dd_p)

        def stage_C(st, host):
            """dw2 MMs (3 paired DR passes over u0,u1) into the d2 half of
            the CURRENT iteration's dd_p (host st)."""
            wC = st["wE"]  # d2 computed at the same width as the front ops
            u = st["u"]
            dd_p = host["dd_p"]
            for k, pk in enumerate((P_D2K0, P_D2K1, P_D2K2)):
                nc.tensor.matmul(dd_p[:, 512:512 + wC], wpair(pk),
                                 _planes(u, k, wC, 516),
                                 start=(k == 0), stop=(k == 2), perf_mode=DR)
            st.update(dd_host=host)

        def stage_W(st_a, st_c):
            """One wide relu evac covering d1p(i) and d2p(i-LAG); narrow
            single evacs at the pipeline edges. Rotates DVE/ACT."""
            if st_a is not None:
                dd = work.tile([C, 1032], F8, tag="dd", bufs=4)
                st_a.update(dd=dd)
            if st_a is not None and st_c is not None and st_a["wE"] == 512 \
                    and st_c["wE"] == 512:
                dd_p = st_a["dd_p"]
                out_ap = _planes(dd, 0, 512, 516)
                in_ap = _planes(dd_p, 0, 512, 512)
                if st_a["e_w"] == "A":
                    nc.scalar.activation(out_ap, in_ap, Relu)
                else:
                    nc.vector.tensor_scalar_max(out_ap, in_ap, 0.0)
                st_c.update(d2s=dd)
                return
            if st_a is not None:
                wE = st_a["wE"]
                relu_to("A", dd[:, :wE], st_a["dd_p"][:, :wE])
            if st_c is not None:
                wC = st_c["wE"]
                host = st_c["dd_host"]
                dd_c = host.get("dd")
                relu_to("A", dd_c[:, 516:516 + wC],
                        host["dd_p"][:, 512:512 + wC])
                st_c.update(d2s=dd_c)

        def stage_B(st):
            """b1p + a MMs into the 2-bank ba tile -> combo STT -> u
            (u0 = relu(b1p)*m, u1 = relu(a)*m in one DVE op; the mask AP
            broadcasts with plane-stride 0)."""
            g = cur[st["gi"]]
            la, wE = st["la"], st["wE"]
            ba = ps.tile([C, 1024], F32, tag="ba", name="ba")
            nc.tensor.matmul(ba[:, :wE], wpair(P_W1),
                             _dr_rhs(st["dd"], 0, wE),
                             start=True, stop=True, perf_mode=DR)
            nc.tensor.matmul(ba[:, 512:512 + wE], wpair(P_W21),
                             _planes(g["xs"], la + 1, wE, S),
                             start=True, stop=True, perf_mode=DR)
            u = work.tile([C, 1032], F8, tag="u", bufs=4)
            out_ap = _planes(u, 0, wE, 516)
            in_ap = _planes(ba, 0, wE, 512)
            mb = g["ms"][:, la:la + wE]
            m_ap = AP(mb.tensor, mb.offset,
                      [list(mb.ap[0]), [0, 2], [1, wE]])
            nc.vector.scalar_tensor_tensor(
                out_ap, in_ap, 0.0, m_ap,
                mybir.AluOpType.max, mybir.AluOpType.mult)
            st.update(u=u)

        def stage_D1(st):
            """b2p MM -> b2r relu. The fusion MMs run one iteration later
            (stage_D2) so no PE instruction ever waits on a same-iteration
            evacuation."""
            dd_c = st["d2s"]
            P_ = st["P_"]
            b2p = ps.tile([C, 512], F32, tag="b2", name="b2p")
            nc.tensor.matmul(b2p[:, :P_], wpair(P_W2),
                             _dr_rhs(dd_c, 516, P_),
                             start=True, stop=True, perf_mode=DR)
            b2r = work.tile([C, 516], F8, tag="b2r", bufs=3)
            relu_to(st["e_b2r"], b2r[:, :P_], b2p[:, :P_])
            st.update(b2r=b2r)

        def stage_D2(st):
            """fusion MMs; Wfb first (its b2r is one iteration old), then
            Wfh + the x/Wfh residual planes close the group."""
            g = cur[st["gi"]]
            la, P_ = st["la"], st["P_"]
            fp = ps.tile([C, 512], F32, tag="f", name="fp")
            nc.tensor.matmul(fp[:, :P_], wpair(P_WFB),
                             _dr_rhs(st["b2r"], 0, P_),
                             start=True, stop=False, perf_mode=DR)
            nc.tensor.matmul(fp[:, :P_], wpair(P_WFH),
                             _planes(g["xs"], la + 2, P_, S),
                             start=False, stop=False, perf_mode=DR)
            # fp8 residual of the Wfh weight itself: the direct h->out path
            # is unattenuated, so Wfh's own quantization error (~2% rel)
            # must be compensated too.
            nc.tensor.matmul(fp[:, :P_], wpair(P_WFHR),
                             _planes(g["xs"], la + 2, P_, S),
                             start=False, stop=True, perf_mode=DR)
            st.update(fp=fp)

        def stage_E(st):
            """final relu + output flush bookkeeping."""
            g = cur[st["gi"]]
            la, P_ = st["la"], st["P_"]
            relu_to(st["e_out"], g["o_c"][:, la:la + P_], st["fp"][:, :P_])
            g["done"] += 1
            ntiles = len(groups[st["gi"]])
            # flush every 2 finished tiles
            if g["done"] % 2 == 0 or g["done"] == ntiles:
                lo, hi = g["flushed"], la + P_
                nc.sync.dma_start(out=y[:, g["ga"] + lo:g["ga"] + hi],
                                  in_=g["o_c"][:, lo:hi])
                g["flushed"] = hi

        flat = [t_ for g in groups for t_ in g]
        n = len(flat)
        sts = []
        loaded = 0

        def ensure_loaded(upto):
            nonlocal loaded
            while loaded <= min(upto, len(groups) - 1):
                load_group(loaded)
                loaded += 1

        # Pipeline skew (iteration i):
        #   A(i) dw1 -> dd_p(i)[0:512]
        #   C(i-LAG) dw2 -> dd_p(i)[512:1024]   (u from B(i-LAG), 2 iters old)
        #   W(i) wide relu dd_p(i) -> d1s(i), d2s(i-LAG)
        #   B(i-2) b1p/a MMs + combo STT -> u
        #   E(i-LAG-4) out relu + flush   (before D2: frees the f bank)
        #   D2(i-LAG-3) fusion MMs        (b2r one iteration old)
        #   D1(i-LAG-2) b2p MM + b2r relu
        LAG = 4
        ensure_loaded(0)
        for i in range(n + LAG + 4):
            st_a = None
            if i < n:
                a, P_ = flat[i]
                gi = group_of[a]
                ensure_loaded(gi + 1)
                # LP balance: DVE {combo + single on 5-of-6 tiles} ~1739,
                # ACT {dd-wide + singles} ~1753; on the 1-in-6 "light"
                # tile DVE runs only the combo and ACT absorbs all three
                # remaining ops (the 2-iteration slack edges soak it up).
                light = (i % 6 == 5)
                st = dict(a=a, P_=P_, wE=P_ + 2, gi=gi,
                          la=a - cur[gi]["ga"],
                          e_w="A",
                          e_b2r=("A" if light else "D"),
                          e_out="A")
                sts.append(st)
                st_a = st
                stage_A(st)
            st_c = sts[i - LAG] if 0 <= i - LAG < n else None
            if st_c is not None:
                host = st_a if st_a is not None else None
                if host is None:
                    # drain iterations: d2 half still needs a PSUM pair;
                    # allocate a fresh one for the trailing tiles
                    dd_p_t = ps.tile([C, 1024], F32, tag="ddp", bufs=2,
                                     name="dd_p_t")
                    dd_t = work.tile([C, 1032], F8, tag="dd", bufs=4,
                                     name="dd_t")
                    host = dict(dd_p=dd_p_t, dd=dd_t)
                stage_C(st_c, host)
            stage_W(st_a, st_c)
            if 0 <= i - 2 < n:
                stage_B(sts[i - 2])
            if 0 <= i - LAG - 4 < n:
                stage_E(sts[i - LAG - 4])
            if 0 <= i - LAG - 3 < n:
                stage_D2(sts[i - LAG - 3])
            if 0 <= i - LAG - 2 < n:
                stage_D1(sts[i - LAG - 2])

    nc.compile()
    return nc


def kernel(x, w_b1_dw, w_b1_pw, w_b2_1x1, w_b2_dw, w_b2_pw, w_fusion):
    global LAST_RESULT, _nc_cache

    x = np.asarray(x, dtype=np.float32)
    h = np.ascontiguousarray(x.T)
    mask = _mask_cn()

    # host-side shard prep: [C, N] layouts, zero-padded halos; second
    # plane is the fp8 quantization residual of x (see _build_nc notes)
    x8_pad = np.zeros((C, N + 8), dtype=NP8)
    x8_pad[:, 2:N + 2] = h.astype(NP8)
    r8_pad = np.zeros((C, N + 8), dtype=NP8)
    r8_pad[:, 2:N + 2] = (h - x8_pad[:, 2:N + 2].astype(np.float32)
                          ).astype(NP8)
    mk_pad = np.zeros((C, N + 2), dtype=NP8)
    mk_pad[:, 1:N + 1] = mask.astype(NP8)

    def taps(wdw):  # [C,1,3,3] -> per-channel taps along N
        return np.asarray(wdw)[:, 0, :, 1]  # [C, 3]

    t1 = taps(w_b1_dw)
    t2 = taps(w_b2_dw)

    def diag8(v):
        return np.diag(v.astype(np.float32)).astype(NP8)

    def lhsT8(w):  # [O, I] -> [I, O] fp8
        return np.ascontiguousarray(np.asarray(w, dtype=np.float32).T).astype(NP8)

    zero = np.zeros((C, C), dtype=NP8)
    w21T = lhsT8(np.asarray(w_b2_1x1)[:, :, 0, 0])
    wfh_f32 = np.ascontiguousarray(
        np.asarray(w_fusion, dtype=np.float32)[:, :C, 0, 0].T)
    wfhT = wfh_f32.astype(NP8)
    wfhrT = (wfh_f32 - wfhT.astype(np.float32)).astype(NP8)
    pairs = [
        (diag8(t1[:, 0]), diag8(t1[:, 2])),
        (diag8(t1[:, 1]), diag8(t1[:, 1])),   # second plane: x residual
        (lhsT8(np.asarray(w_b1_pw)[:, :, 0, 0]), zero),
        (w21T, w21T),                          # second plane: x residual
        (diag8(t2[:, 0]), diag8(t2[:, 0])),
        (diag8(t2[:, 1]), diag8(t2[:, 1])),
        (diag8(t2[:, 2]), diag8(t2[:, 2])),
        (lhsT8(np.asarray(w_b2_pw)[:, :, 0, 0]), zero),
        (wfhT, wfhT),                          # second plane: x residual
        (lhsT8(np.asarray(w_fusion)[:, C:, 0, 0]), zero),
        (wfhrT, wfhrT),                        # Wfh weight residual
    ]
    w8_host = np.empty((C, NPAIR * 2 * C), dtype=NP8)
    for k, (p0, p1) in enumerate(pairs):
        w8_host[:, (2 * k) * C:(2 * k + 1) * C] = p0
        w8_host[:, (2 * k + 1) * C:(2 * k + 2) * C] = p1

    in_maps = []
    for i in range(NCORES):
        s = i * NSH
        x8_core = np.empty((C, 2 * (NSH + 8)), dtype=NP8)
        x8_core[:, :NSH + 8] = x8_pad[:, s:s + NSH + 8]
        x8_core[:, NSH + 8:] = r8_pad[:, s:s + NSH + 8]
        in_maps.append({
            "x8": x8_core,
            "mk": np.ascontiguousarray(mk_pad[:, s:s + NSH + 2]),
            "w8": w8_host,
        })

    if _nc_cache is None:
        _nc_cache = _build_nc()

    res = run_bass_kernel_spmd(
        _nc_cache, in_maps, core_ids=list(range(NCORES)), trace=TRACE
    )
    LAST_RESULT = res

    out = np.empty((C, N), dtype=np.float32)
    for i in range(NCORES):
        out[:, i * NSH:(i + 1) * NSH] = res.results[i]["y"].astype(np.float32)
    return np.ascontiguousarray(out.T)


# revision 61
# speedup vs baseline: 1.0178x; 1.0057x over previous
"""Trainium2 Bass kernel for nn_KB_Mapping_19361712570541 (dense_cnn).

Math (W=1 image dim folded away; h = x.T in [C, N] channels-on-partition):
  dw3(h, w)[c,n] = w0[c]h[c,n-1] + w1[c]h[c,n] + w2[c]h[c,n+1]   (zero pad)
  b1  = relu(W1pw @ relu(dw3(h, wd1)))
  b2  = (relu(W21x1 @ h) + b1) * mask
  b2  = relu(W2pw @ relu(dw3(b2, wd2)))
  out = relu(Wf[:, :C] @ h + Wf[:, C:] @ b2)          -> out.T is [N, C]

Sharding: data-parallel along N across 8 cores; each core's input slab
carries halos of x/mask so no cross-core communication is needed.

Implementation notes (cost-model driven):
- The kernel is PSUM-evacuation bound: every relu forces one PSUM->SBUF
  pointwise op, PSUM is only reachable from DVE (658ns/512 cols) and
  ACT (612ns/512; 1039ns/1024 across two banks) - GPSIMD cannot access
  PSUM, and fp32 PSUM operands disable all DVE fast modes.
- The mask multiply is FOLDED into the relu evacuations that feed dw2:
  u0 = relu(b1p)*m, u1 = relu(a)*m (valid since m>=0); dw2 consumes
  u0+u1 linearly via three paired DoubleRow passes (plane stride 516
  inside one SBUF tile). b1p and a are matmul'd into ONE [C,1024] PSUM
  tile (2 adjacent banks) and evacuated by a single DVE
  scalar_tensor_tensor whose input AP spans both banks, output planes
  stride 516 into u, and whose mask AP uses plane-stride 0 (broadcast).
- d1p(j) and d2p(j-2) (independent pipeline stages) share one
  double-buffered [C,1024] PSUM pair and are evacuated by ONE wide ACT
  relu into a shared dd tile (d1s at [0:516], d2s at [516:1032]).
- Per tile: DVE = combo 1192 + 5/6 x 658 (b2r), ACT = wide 1039 +
  612 (out) + 1/6 x 612 -> ~1.75us/tile on both engines.
- All matmuls are fp8e4 DoubleRow (cost = out cols x 0.208ns). x ships
  ONCE as fp8 (fusion's Wfh@h term also fp8; rel err ~5e-3 vs 2e-2
  gate), mask as fp8, y as fp16: ~25us DMA per core.
- PSUM banks: dd_p x2 pairs (4), ba pair (2), b2 (1), f (1) = 8.
  Zero-weight DR planes read 2 cols past valid data; work-tile rings
  are memset once per buffer at startup (interp faults on uninit).
- Software-pipelined emission, iteration i:
  A(i) dw1 MMs; C(i-2) dw2 MMs; W(i) wide dd evac; B(i-1) b1p/a MMs +
  combo; E(i-4) out relu + flush; D(i-3) b2p MM + b2r + fusion MMs.
  (E before D so the single-bank f is freed before the next fusion.)
"""

import numpy as np
from contextlib import ExitStack

import ml_dtypes

import concourse.bass as bass
import concourse.bacc as bacc
import concourse.tile as tile
import concourse.mybir as mybir
from concourse.ap import AP
from concourse.bass_utils import run_bass_kernel_spmd

C = 128
N = 131072
NCORES = 8
NSH = N // NCORES          # 16384 output columns per core
T = 510                    # steady-state tile width (wE = 512 = one PSUM bank)
MASK_SEED = 42
MASK_P = 0.5

F32 = mybir.dt.float32
F16 = mybir.dt.float16
F8 = mybir.dt.float8e4
NP8 = ml_dtypes.float8_e4m3
DR = mybir.MatmulPerfMode.DoubleRow
Relu = mybir.ActivationFunctionType.Relu

# DR weight-pair indices in w8 (each pair is [C, 2, C] -> 256 cols)
(P_D1A, P_D1B, P_W1, P_W21, P_D2K0, P_D2K1, P_D2K2, P_W2, P_WFH,
 P_WFB, P_WFHR) = range(11)
NPAIR = 11

LAST_RESULT = None         # BassKernelResults of the most recent run (for test.py)
TRACE = False

_mask_cache = None
_nc_cache = None


def _mask_cn() -> np.ndarray:
    """The reference's fixed Bernoulli mask in [C, N] layout, float32."""
    global _mask_cache
    if _mask_cache is None:
        import jax
        cpu = jax.devices("cpu")[0]
        with jax.default_device(cpu):
            m = jax.random.bernoulli(
                jax.random.key(MASK_SEED), 1.0 - MASK_P, (1, C, N, 1)
            )
            _mask_cache = np.asarray(m)[0, :, :, 0].astype(np.float32)
    return _mask_cache


def _tiles():
    """(a, width) list covering [0, NSH); narrow leader fills the pipe and
    a narrow trailer drains it."""
    widths = [256] + [T] * 31 + [318]
    assert sum(widths) == NSH
    out, a = [], 0
    for w in widths:
        out.append((a, w))
        a += w
    return out


def _groups(tiles):
    """Graduated DMA groups as slices of the tile list."""
    sizes = [4, 5, 7, 8, 9]
    gs, i = [], 0
    for s in sizes:
        if i >= len(tiles):
            break
        gs.append(tiles[i:i + s])
        i += s
    if i < len(tiles):
        gs.append(tiles[i:])
    return gs


def _dr_rhs(t, col, n, delta=2):
    """[C, 2, n] moving AP over tile t: plane0 at col, plane1 at col+delta."""
    base = t[:, col:col + n]
    return AP(base.tensor, base.offset,
              [list(base.ap[0]), [delta, 2], [1, n]])


def _planes(t, col, n, stride):
    """[C, 2, n] AP over tile t: planes at col and col+stride."""
    base = t[:, col:col + n]
    return AP(base.tensor, base.offset,
              [list(base.ap[0]), [stride, 2], [1, n]])


def _build_nc():
    nc = bacc.Bacc("TRN2", target_bir_lowering=False)

    # x8 holds the fp8 input AND its fp8 quantization residual as a second
    # plane: x8[:, 0:NSH+8] = fp8(x), x8[:, NSH+8:] = fp8(x - fp8(x)).
    # The residual rides the otherwise-zero second weight plane of the
    # fusion / b2a / dw1-t1 DoubleRow passes, recovering ~fp16 accuracy on
    # the direct paths at zero extra PE cost.
    x8 = nc.dram_tensor("x8", [C, 2 * (NSH + 8)], F8, kind="ExternalInput")
    mk = nc.dram_tensor("mk", [C, NSH + 2], F8, kind="ExternalInput")
    w8 = nc.dram_tensor("w8", [C, NPAIR * 2 * C], F8, kind="ExternalInput")
    y = nc.dram_tensor("y", [C, NSH], F16, kind="ExternalOutput")

    tiles = _tiles()
    groups = _groups(tiles)
    gw_max = max(sum(w for _, w in g) for g in groups)
    S = gw_max + 6            # xs slab plane stride (x -> residual)
    group_of = {}
    for gi, g in enumerate(groups):
        for t_ in g:
            group_of[t_[0]] = gi

    with ExitStack() as ctx:
        tc = ctx.enter_context(tile.TileContext(nc))
        wpool = ctx.enter_context(tc.tile_pool(name="weights", bufs=1))
        slab = ctx.enter_context(tc.tile_pool(name="slab", bufs=3))
        opool = ctx.enter_context(tc.tile_pool(name="out", bufs=3))
        work = ctx.enter_context(tc.tile_pool(name="work", bufs=3))
        ps = ctx.enter_context(tc.tile_pool(name="ps", bufs=1, space="PSUM"))

        w8_sb = wpool.tile([C, NPAIR * 2 * C], F8)

        def wpair(k):
            return w8_sb[:, k * 2 * C:(k + 1) * 2 * C].rearrange(
                "p (two m) -> p two m", two=2)

        def relu_to(eng, dst, src):
            if eng == "A":
                nc.scalar.activation(dst, src, Relu)
            elif eng == "S":
                # drain helper: both engines in parallel on half-widths
                w = dst.shape[-1]
                h = min(int(w * 0.51), w)
                nc.vector.tensor_scalar_max(dst[:, :h], src[:, :h], 0.0)
                nc.scalar.activation(dst[:, h:], src[:, h:], Relu)
            else:
                nc.vector.tensor_scalar_max(dst, src, 0.0)

        # One-time zero of every ring buffer that zero-weight DR planes or
        # edge-width wide evacs read past valid data on (the interpreter
        # hard-faults on uninitialized reads). Runs on Pool/DVE during the
        # initial DMA fill, off the critical path.
        # preload the ACT Relu table before any real data arrives: the
        # one-time LoadActFuncSet (1283ns) otherwise blocks the first
        # wide evacuation mid-fill.
        warm = wpool.tile([C, 4], F16)
        nc.gpsimd.memset(warm[:, :], 0.0)
        nc.scalar.activation(warm[:, 2:4], warm[:, 0:2], Relu)

        ZRINGS = (("dd", 4, 1032), ("u", 4, 1032), ("b2r", 3, 516))
        ztiles = {ztag: [work.tile([C, zw], F8, tag=ztag, bufs=zb,
                                   name=f"z_{ztag}")
                         for _ in range(zb)]
                  for ztag, zb, zw in ZRINGS}
        # earliest-used buffers first, split across Pool and DVE so the
        # first tiles' writes aren't serialized behind one engine's queue
        zi = 0
        for bi in range(3):
            for ztag, zb, zw in ZRINGS:
                if bi < zb:
                    zt = ztiles[ztag][bi]
                    if zi % 2 == 0:
                        nc.gpsimd.memset(zt[:, :], 0.0)
                    else:
                        nc.vector.memset(zt[:, :], 0.0)
                    zi += 1

        # per-group slab state
        cur = {}

        def load_group(gi):
            g = groups[gi]
            ga = g[0][0]
            gw = sum(w for _, w in g)
            xs = slab.tile([C, 2 * S], F8, tag="xs")
            nc.sync.dma_start(
                out=_planes(xs, 0, gw + 6, S),
                in_=_planes(x8, ga, gw + 6, NSH + 8))
            ms = slab.tile([C, gw_max + 2], F8, tag="ms")
            if gi == 0:
                # fill-critical order: dw1 pairs -> first two tiles' mask
                # -> W1/W21 pairs -> rest of mask -> back-pipeline pairs
                # (dw2/W2/Wf, needed only ~LAG iterations in).
                nc.sync.dma_start(out=w8_sb[:, :512], in_=w8[:, :512])
                nc.sync.dma_start(out=ms[:, :772], in_=mk[:, :772])
                nc.sync.dma_start(out=w8_sb[:, 512:2048],
                                  in_=w8[:, 512:2048])
                nc.sync.dma_start(out=ms[:, 772:gw + 2],
                                  in_=mk[:, 772:gw + 2])
                nc.sync.dma_start(out=w8_sb[:, 2048:], in_=w8[:, 2048:])
            else:
                nc.sync.dma_start(out=ms[:, :gw + 2],
                                  in_=mk[:, ga:ga + gw + 2])
            o_c = opool.tile([C, gw_max], F16, tag="oc")
            cur[gi] = dict(ga=ga, gw=gw, xs=xs, ms=ms, o_c=o_c,
                           flushed=0, done=0)

        def stage_A(st):
            """dw1 MMs into the d1 half of this tile's dd_p pair."""
            g = cur[st["gi"]]
            la, wE = st["la"], st["wE"]
            xs = g["xs"]
            dd_p = ps.tile([C, 1024], F32, tag="ddp", bufs=2, name="dd_p")
            nc.tensor.matmul(dd_p[:, :wE], wpair(P_D1A),
                             _dr_rhs(xs, la, wE),
                             start=True, stop=False, perf_mode=DR)
            nc.tensor.matmul(dd_p[:, :wE], wpair(P_D1B),
                             _planes(xs, la + 1, wE, S),
                             start=False, stop=True, perf_mode=DR)
            st.update(dd_p=dd_p)

        def stage_C(st, host):
            """dw2 MMs (3 paired DR passes over u0,u1) into the d2 half of
            the CURRENT iteration's dd_p (host st)."""
            wC = st["wE"]  # d2 computed at the same width as the front ops
            u = st["u"]
            dd_p = host["dd_p"]
            for k, pk in enumerate((P_D2K0, P_D2K1, P_D2K2)):
                nc.tensor.matmul(dd_p[:, 512:512 + wC], wpair(pk),
                                 _planes(u, k, wC, 516),
                                 start=(k == 0), stop=(k == 2), perf_mode=DR)
            st.update(dd_host=host)

        def stage_W(st_a, st_c):
            """One wide relu evac covering d1p(i) and d2p(i-LAG); narrow
            single evacs at the pipeline edges. Rotates DVE/ACT."""
            if st_a is not None:
                dd = work.tile([C, 1032], F8, tag="dd", bufs=4)
                st_a.update(dd=dd)
            if st_a is not None and st_c is not None and st_a["wE"] == 512 \
                    and st_c["wE"] == 512:
                dd_p = st_a["dd_p"]
                out_ap = _planes(dd, 0, 512, 516)
                in_ap = _planes(dd_p, 0, 512, 512)
                if st_a["e_w"] == "A":
                    nc.scalar.activation(out_ap, in_ap, Relu)
                else:
                    nc.vector.tensor_scalar_max(out_ap, in_ap, 0.0)
                st_c.update(d2s=dd)
                return
            if st_a is not None:
                wE = st_a["wE"]
                relu_to("A", dd[:, :wE], st_a["dd_p"][:, :wE])
            if st_c is not None:
                wC = st_c["wE"]
                host = st_c["dd_host"]
                dd_c = host.get("dd")
                relu_to("A", dd_c[:, 516:516 + wC],
                        host["dd_p"][:, 512:512 + wC])
                st_c.update(d2s=dd_c)

        def stage_B(st):
            """b1p + a MMs into the 2-bank ba tile -> combo STT -> u
            (u0 = relu(b1p)*m, u1 = relu(a)*m in one DVE op; the mask AP
            broadcasts with plane-stride 0)."""
            g = cur[st["gi"]]
            la, wE = st["la"], st["wE"]
            ba = ps.tile([C, 1024], F32, tag="ba", name="ba")
            nc.tensor.matmul(ba[:, :wE], wpair(P_W1),
                             _dr_rhs(st["dd"], 0, wE),
                             start=True, stop=True, perf_mode=DR)
            nc.tensor.matmul(ba[:, 512:512 + wE], wpair(P_W21),
                             _planes(g["xs"], la + 1, wE, S),
                             start=True, stop=True, perf_mode=DR)
            u = work.tile([C, 1032], F8, tag="u", bufs=4)
            out_ap = _planes(u, 0, wE, 516)
            in_ap = _planes(ba, 0, wE, 512)
            mb = g["ms"][:, la:la + wE]
            m_ap = AP(mb.tensor, mb.offset,
                      [list(mb.ap[0]), [0, 2], [1, wE]])
            nc.vector.scalar_tensor_tensor(
                out_ap, in_ap, 0.0, m_ap,
                mybir.AluOpType.max, mybir.AluOpType.mult)
            st.update(u=u)

        def stage_D1(st):
            """b2p MM -> b2r relu. The fusion MMs run one iteration later
            (stage_D2) so no PE instruction ever waits on a same-iteration
            evacuation."""
            dd_c = st["d2s"]
            P_ = st["P_"]
            b2p = ps.tile([C, 512], F32, tag="b2", name="b2p")
            nc.tensor.matmul(b2p[:, :P_], wpair(P_W2),
                             _dr_rhs(dd_c, 516, P_),
                             start=True, stop=True, perf_mode=DR)
            b2r = work.tile([C, 516], F8, tag="b2r", bufs=3)
            relu_to(st["e_b2r"], b2r[:, :P_], b2p[:, :P_])
            st.update(b2r=b2r)

        def stage_D2(st):
            """fusion MMs; Wfb first (its b2r is one iteration old), then
            Wfh + the x/Wfh residual planes close the group."""
            g = cur[st["gi"]]
            la, P_ = st["la"], st["P_"]
            fp = ps.tile([C, 512], F32, tag="f", name="fp")
            nc.tensor.matmul(fp[:, :P_], wpair(P_WFB),
                             _dr_rhs(st["b2r"], 0, P_),
                             start=True, stop=False, perf_mode=DR)
            nc.tensor.matmul(fp[:, :P_], wpair(P_WFH),
                             _planes(g["xs"], la + 2, P_, S),
                             start=False, stop=False, perf_mode=DR)
            # fp8 residual of the Wfh weight itself: the direct h->out path
            # is unattenuated, so Wfh's own quantization error (~2% rel)
            # must be compensated too.
            nc.tensor.matmul(fp[:, :P_], wpair(P_WFHR),
                             _planes(g["xs"], la + 2, P_, S),
                             start=False, stop=True, perf_mode=DR)
            st.update(fp=fp)

        def stage_E(st):
            """final relu + output flush bookkeeping."""
            g = cur[st["gi"]]
            la, P_ = st["la"], st["P_"]
            relu_to(st["e_out"], g["o_c"][:, la:la + P_], st["fp"][:, :P_])
            g["done"] += 1
            ntiles = len(groups[st["gi"]])
            # flush every 2 finished tiles
            if g["done"] % 2 == 0 or g["done"] == ntiles:
                lo, hi = g["flushed"], la + P_
                nc.sync.dma_start(out=y[:, g["ga"] + lo:g["ga"] + hi],
                                  in_=g["o_c"][:, lo:hi])
                g["flushed"] = hi

        flat = [t_ for g in groups for t_ in g]
        n = len(flat)
        sts = []
        loaded = 0

        def ensure_loaded(upto):
            nonlocal loaded
            while loaded <= min(upto, len(groups) - 1):
                load_group(loaded)
                loaded += 1

        # Pipeline skew (iteration i):
        #   A(i) dw1 -> dd_p(i)[0:512]
        #   C(i-LAG) dw2 -> dd_p(i)[512:1024]   (u from B(i-LAG), 2 iters old)
        #   W(i) wide relu dd_p(i) -> d1s(i), d2s(i-LAG)
        #   B(i-2) b1p/a MMs + combo STT -> u
        #   E(i-LAG-4) out relu + flush   (before D2: frees the f bank)
        #   D2(i-LAG-3) fusion MMs        (b2r one iteration old)
        #   D1(i-LAG-2) b2p MM + b2r relu
        LAG = 4
        ensure_loaded(0)
        for i in range(n + LAG + 4):
            st_a = None
            if i < n:
                a, P_ = flat[i]
                gi = group_of[a]
                ensure_loaded(gi + 1)
                # LP balance: DVE {combo + single on 5-of-6 tiles} ~1739,
                # ACT {dd-wide + singles} ~1753; on the 1-in-6 "light"
                # tile DVE runs only the combo and ACT absorbs all three
                # remaining ops (the 2-iteration slack edges soak it up).
                light = (i % 6 == 5)
                st = dict(a=a, P_=P_, wE=P_ + 2, gi=gi,
                          la=a - cur[gi]["ga"],
                          e_w="A",
                          e_b2r=("A" if light else "D"),
                          e_out="A")
                sts.append(st)
                st_a = st
                stage_A(st)
            st_c = sts[i - LAG] if 0 <= i - LAG < n else None
            if st_c is not None:
                host = st_a if st_a is not None else None
                if host is None:
                    # drain iterations: d2 half still needs a PSUM pair;
                    # allocate a fresh one for the trailing tiles
                    dd_p_t = ps.tile([C, 1024], F32, tag="ddp", bufs=2,
                                     name="dd_p_t")
                    dd_t = work.tile([C, 1032], F8, tag="dd", bufs=4,
                                     name="dd_t")
                    host = dict(dd_p=dd_p_t, dd=dd_t)
                stage_C(st_c, host)
            stage_W(st_a, st_c)
            if 0 <= i - 2 < n:
                stage_B(sts[i - 2])
            if 0 <= i - LAG - 4 < n:
                stage_E(sts[i - LAG - 4])
            if 0 <= i - LAG - 3 < n:
                stage_D2(sts[i - LAG - 3])
            if 0 <= i - LAG - 2 < n:
                stage_D1(sts[i - LAG - 2])

    nc.compile()
    return nc


def kernel(x, w_b1_dw, w_b1_pw, w_b2_1x1, w_b2_dw, w_b2_pw, w_fusion):
    global LAST_RESULT, _nc_cache

    x = np.asarray(x, dtype=np.float32)
    h = np.ascontiguousarray(x.T)
    mask = _mask_cn()

    # host-side shard prep: [C, N] layouts, zero-padded halos; second
    # plane is the fp8 quantization residual of x (see _build_nc notes)
    x8_pad = np.zeros((C, N + 8), dtype=NP8)
    x8_pad[:, 2:N + 2] = h.astype(NP8)
    r8_pad = np.zeros((C, N + 8), dtype=NP8)
    r8_pad[:, 2:N + 2] = (h - x8_pad[:, 2:N + 2].astype(np.float32)
                          ).astype(NP8)
    mk_pad = np.zeros((C, N + 2), dtype=NP8)
    mk_pad[:, 1:N + 1] = mask.astype(NP8)

    def taps(wdw):  # [C,1,3,3] -> per-channel taps along N
        return np.asarray(wdw)[:, 0, :, 1]  # [C, 3]

    t1 = taps(w_b1_dw)
    t2 = taps(w_b2_dw)

    def diag8(v):
        return np.diag(v.astype(np.float32)).astype(NP8)

    def lhsT8(w):  # [O, I] -> [I, O] fp8
        return np.ascontiguousarray(np.asarray(w, dtype=np.float32).T).astype(NP8)

    zero = np.zeros((C, C), dtype=NP8)
    w21T = lhsT8(np.asarray(w_b2_1x1)[:, :, 0, 0])
    wfh_f32 = np.ascontiguousarray(
        np.asarray(w_fusion, dtype=np.float32)[:, :C, 0, 0].T)
    wfhT = wfh_f32.astype(NP8)
    wfhrT = (wfh_f32 - wfhT.astype(np.float32)).astype(NP8)
    pairs = [
        (diag8(t1[:, 0]), diag8(t1[:, 2])),
        (diag8(t1[:, 1]), diag8(t1[:, 1])),   # second plane: x residual
        (lhsT8(np.asarray(w_b1_pw)[:, :, 0, 0]), zero),
        (w21T, w21T),                          # second plane: x residual
        (diag8(t2[:, 0]), diag8(t2[:, 0])),
        (diag8(t2[:, 1]), diag8(t2[:, 1])),
        (diag8(t2[:, 2]), diag8(t2[:, 2])),
        (lhsT8(np.asarray(w_b2_pw)[:, :, 0, 0]), zero),
        (wfhT, wfhT),                          # second plane: x residual
        (lhsT8(np.asarray(w_fusion)[:, C:, 0, 0]), zero),
        (wfhrT, wfhrT),                        # Wfh weight residual
    ]
    w8_host = np.empty((C, NPAIR * 2 * C), dtype=NP8)
    for k, (p0, p1) in enumerate(pairs):
        w8_host[:, (2 * k) * C:(2 * k + 1) * C] = p0
        w8_host[:, (2 * k + 1) * C:(2 * k + 2) * C] = p1

    in_maps = []
    for i in range(NCORES):
        s = i * NSH
        x8_core = np.empty((C, 2 * (NSH + 8)), dtype=NP8)
        x8_core[:, :NSH + 8] = x8_pad[:, s:s + NSH + 8]
        x8_core[:, NSH + 8:] = r8_pad[:, s:s + NSH + 8]
        in_maps.append({
            "x8": x8_core,
            "mk": np.ascontiguousarray(mk_pad[:, s:s + NSH + 2]),
            "w8": w8_host,
        })

    if _nc_cache is None:
        _nc_cache = _build_nc()

    res = run_bass_kernel_spmd(
        _nc_cache, in_maps, core_ids=list(range(NCORES)), trace=TRACE
    )
    LAST_RESULT = res

    out = np.empty((C, N), dtype=np.float32)
    for i in range(NCORES):
        out[:, i * NSH:(i + 1) * NSH] = res.results[i]["y"].astype(np.float32)
    return np.ascontiguousarray(out.T)
